# revision 55
# baseline (speedup 1.0000x reference)
"""Multi-head self-attention Trainium2 kernel (8 NeuronCores, SPMD).

Problem: B=2, S=2048, D=1024, H=16, Dk=64; torch-style Linear projections
(x @ W.T + b), custom softmax: p = exp(scores/8), attn = p / (sum(p) + 1e-8).

Sharding: 32 (batch, head) pairs over 8 cores -> core c handles batch c//4,
heads [4*(c%4), 4*(c%4)+4). Each core projects only its 256 features of
q/k/v; attention is embarrassingly parallel over (b, h).

All matmul operands are bf16 (same PE rate as fp32r at these shapes, but
half the DMA traffic and LDWEIGHTS time; fp32 accumulation in PSUM keeps
the contractions exact). The PE is the bottleneck engine, so the whole
kernel is ONE continuous PE instruction stream with no phase barriers.

AV formulation (the big win vs the 191us version): instead of
ctx^T[65,512] = v[128t,65]^T @ pT[128t,512] (M=65 wastes half the PE
columns: 232ns per 512-row matmul, measured), attention-times-V runs as
ctx[s,d]: ctx[128s,65] += pT[128t,128s]^T @ v65[128t,65] with the exp'd
scores as the STATIONARY operand (full M=128). A fresh 128x128 bf16
LDWEIGHTS per 65-row matmul sustains 49ns/matmul (measured): the weight
load pipelines under the previous matmul. 8 such matmuls replace each
old 2x512-row step (394ns vs 464ns) AND the ctx lands [s,d]-major, so
the per-block PE transposes, the [65,512] PSUM->SBUF copies, and their
ACT/DVE load all disappear. The denominator rides along as v's 65th
(ones) column.

Pipeline:
  - prologue: only kT[0]/qT[0] chunk 0 (17 matmuls); everything else is
    deadline-paced filler inside the attention loop
  - 128-step attention loop over 8 (pair, s-chunk) blocks. Per step:
    2 score matmuls (tile_position row-groups run concurrently in the PE
    array), one exp covering both heads into a per-step pT tile (SBUF,
    33-deep pool: pT of block b is consumed by AV chains one block later).
  - AV chains: for block b, 8 chains (s-subtile x head) issue during
    block b+1's steps, one every other step. A chain = 16 matmuls
    accumulating ctx[128,65] in a rotating 4-slot PSUM arena + epilogue:
    DVE reciprocal of the ones-column, Pool scalar_tensor_tensor
    out = ctx * (1/den) + bv straight from PSUM to the SBUF out tile.
    Out tile column-halves DMA (sync queue) as soon as both heads land.
  - exp split across three engines, pattern ADAP per 4 steps: 8/16 on
    ACT (table exp, bf16 out), 4/16 on DVE and 4/16 on Pool via a
    one-instruction Schraudolph in the bf16 bit domain (bits16 =
    trunc(scores*A + B) via f32->u16 convert-on-write, bitcast to bf16;
    ~3% pointwise, mostly cancelled by the sum-normalization).
  - filler: remaining projections (kT[0] c1-3, all of v, qT[0] c1-3, all
    of pair 1) with per-unit deadlines derived from first-use steps; the
    x0/x1 PSUM banks rotate between accumulation chains. v s-tile pairs
    pack two 256-col accumulation groups into one 2KB PSUM bank.
  - DMA layout: the scalar sequencer's issue backlog would gate the first
    exp (DIRECT2D issues are ~0.6us each, in-order with engine ops), so
    qt rides sync (chunk 1 split across both queues), weights ride
    scalar, wv/biases ride gpsimd SWDGE; out rides sync.

Output per core: [2048, 256] fp32 -> host concatenates features per batch.
"""

import sys

sys.path.insert(0, "/opt/trn_rl_repo")

from collections import deque
from contextlib import ExitStack

import ml_dtypes
import numpy as np

import concourse.bass as bass
import concourse.tile as tile
from concourse import bacc, mybir
from concourse.bass_utils import run_bass_kernel_spmd

F32 = mybir.dt.float32
BF16 = mybir.dt.bfloat16
U16 = mybir.dt.uint16

# Schraudolph exp on DVE/Pool, in the bf16 bit domain (the BIR verifier
# forbids int-typed producers feeding fp32r matmuls; bf16 has no such rule):
# exp(g/8) ~= bitcast_bf16(trunc(g*A + B)) via the f32->u16 convert-on-write.
# A = 2^7*log2(e)/8; the down-bias C=5.1 minimizes max pointwise rel err.
SCH_A = float(np.float32((1 << 7) / (8.0 * np.log(2.0))))
SCH_B = float(np.float32((127 << 7) - 5.1))

S = 2048  # sequence length
D = 1024  # d_model
J = 256  # features per core (4 heads x 64)
NKT = 8  # k-tiles of the d_model contraction
NSC = 4  # s-chunks of 512
NTT = 16  # t-tiles of 128
N_CORES = 8

_cached_nc = None
last_result = None  # BassKernelResults of the most recent run (for test.py)


def _build():
    nc = bacc.Bacc(None, target_bir_lowering=False)

    qt = nc.dram_tensor("qt", [D, S], BF16, kind="ExternalInput")
    wq = nc.dram_tensor("wq", [D, J], BF16, kind="ExternalInput")
    wk = nc.dram_tensor("wk", [D, J], BF16, kind="ExternalInput")
    wv = nc.dram_tensor("wv", [D, J], BF16, kind="ExternalInput")
    bq = nc.dram_tensor("bq", [J], F32, kind="ExternalInput")
    bk = nc.dram_tensor("bk", [J], F32, kind="ExternalInput")
    bv = nc.dram_tensor("bv", [J], F32, kind="ExternalInput")
    out = nc.dram_tensor("out", [S, J], F32, kind="ExternalOutput")

    with tile.TileContext(nc) as tc, ExitStack() as ctx:
        wts = ctx.enter_context(tc.tile_pool(name="wts", bufs=1))
        qkp = ctx.enter_context(tc.tile_pool(name="qkp", bufs=1))
        vxp = ctx.enter_context(tc.tile_pool(name="vxp", bufs=1))
        bp = ctx.enter_context(tc.tile_pool(name="bp", bufs=1))
        pTp = ctx.enter_context(tc.tile_pool(name="pTp", bufs=33))
        outp = ctx.enter_context(tc.tile_pool(name="outp", bufs=1))
        qtcp = ctx.enter_context(tc.tile_pool(name="qtc", bufs=1))
        rp = ctx.enter_context(tc.tile_pool(name="rp", bufs=8))
        aps = ctx.enter_context(tc.tile_pool(name="aps", bufs=1, space="PSUM"))
        p1b = ctx.enter_context(tc.tile_pool(name="p1b", bufs=1, space="PSUM"))

        # Weights: 8 k-tiles each of [128, 256], k-major and split across the
        # HWDGE (sync/scalar) / SWDGE (gpsimd) queues, interleaved with the
        # first s-chunk of QT so the first projection matmuls start early
        wq_t = wts.tile([128, NKT, J], BF16, name="wq_t", tag="wq_t")
        wk_t = wts.tile([128, NKT, J], BF16, name="wk_t", tag="wk_t")
        wv_t = wts.tile([128, NKT, J], BF16, name="wv_t", tag="wv_t")
        qtc = [
            qtcp.tile([128, NKT, 512], BF16, name=f"qtc{c}", tag=f"qtc{c}")
            for c in range(NSC)
        ]

        # ALL qtc issues on sync: the scalar sequencer must stay shallow --
        # its DMA-issue backlog (0.6us each, in-order with engine ops) would
        # otherwise gate the first exp
        for k in range(NKT):
            ksl = slice(k * 128, (k + 1) * 128)
            nc.sync.dma_start(qtc[0][:, k, :], qt[ksl, 0:512])
            nc.scalar.dma_start(wk_t[:, k, :], wk[ksl, :])
        for k in range(NKT):
            ksl = slice(k * 128, (k + 1) * 128)
            nc.scalar.dma_start(wq_t[:, k, :], wq[ksl, :])
            nc.gpsimd.dma_start(wv_t[:, k, :], wv[ksl, :])
        # chunk 1 is needed soonest (attention t=4..7 plus v s-tiles 4-7):
        # split it across both queues; chunks 2-3 follow on sync
        for k in range(NKT):
            eng = nc.sync if k % 2 == 0 else nc.scalar
            eng.dma_start(qtc[1][:, k, :], qt[k * 128 : (k + 1) * 128, 512:1024])
        for c in range(2, NSC):
            s0 = c * 512
            for k in range(NKT):
                nc.sync.dma_start(
                    qtc[c][:, k, :], qt[k * 128 : (k + 1) * 128, s0 : s0 + 512]
                )
        # Biases: bq/bk as per-partition scalars [128, 2]; bv broadcast [128, 256]
        bq_t = bp.tile([128, 2], F32, name="bqt")
        nc.gpsimd.dma_start(bq_t[:], bq.rearrange("(m p) -> p m", p=128))
        bk_t = bp.tile([128, 2], F32, name="bkt")
        nc.gpsimd.dma_start(bk_t[:], bk.rearrange("(m p) -> p m", p=128))
        bv_t = bp.tile([128, J], F32, name="bvt")
        bvap = bv[:]
        bv_bcast = bass.AP(
            tensor=bvap.tensor, offset=bvap.offset, ap=[[0, 128], [1, J]]
        )
        nc.gpsimd.dma_start(bv_t[:], bv_bcast)

        scratch = bp.tile([128, 1], F32, name="scratch")

        # Persistent projected tensors
        qT = [qkp.tile([128, S], BF16, name=f"qT{m}", tag=f"qT{m}") for m in range(2)]
        kT = [qkp.tile([128, S], BF16, name=f"kT{m}", tag=f"kT{m}") for m in range(2)]
        v_ext = []
        for t in range(NTT):
            vt = vxp.tile([128, 4, 65], BF16, name=f"vx{t}", tag=f"vx{t}")
            # DVE memsets (it is idle during the prologue; gpsimd's queue is
            # busy streaming wv and would delay the first v epilogues). Only
            # the ones column needs a value; [:, :, 0:64] is fully written by
            # the projection epilogues.
            nc.vector.memset(vt[:, :, 64:65], 1.0)
            v_ext.append(vt)
        out_tiles = [
            outp.tile([128, J], F32, name=f"ot{b}", tag=f"ot{b}") for b in range(16)
        ]

        # pre-load the ACT exp table (it only needs bq_t) so the first
        # attention exp doesn't pay the ~2.7us table-load stall
        nc.scalar.activation(
            scratch[:], bq_t[:, 0:1], mybir.ActivationFunctionType.Exp, scale=0.0
        )

        # ---------- PSUM arenas ----------
        # ctx arena: one PSUM bank, 6 rotating [128, 65] AV accumulation
        # slots (and, during the prologue only, scratch for the first two q
        # projection half-chains -- nothing else touches it that early).
        # proj arena: one PSUM bank, 2 rotating [128, 256] slots. Projection
        # work units are HALF chains (256 wide) so both slots fit one bank;
        # that frees the bank that gives the score tiles a third buffer.
        ctx_arena = aps.tile([128, 512], F32, name="ctxa", tag="ctxa")
        parena = p1b.tile([128, 512], F32, name="parena", tag="parena")
        prot = [0]

        def pslot():
            u = prot[0] % 2
            prot[0] += 1
            return parena[:, 256 * u : 256 * u + 256]

        # ---------- projection instruction chains (half-width units) ------
        def emit_qk_half(pair, c, which, sh, slot_ap=None):
            """8 matmuls + bias epilogue for a 256-col s-half of
            qT/kT[pair][:, c*512+sh*256 : +256]."""
            st = {}
            w_t = wq_t if which == "q" else wk_t
            dst = (qT if which == "q" else kT)[pair]
            b_t = bq_t if which == "q" else bk_t
            s0 = c * 512 + sh * 256

            def mm(k):
                def f():
                    if k == 0:
                        st["t"] = slot_ap if slot_ap is not None else pslot()
                    nc.tensor.matmul(
                        st["t"],
                        w_t[:, k, pair * 128 : (pair + 1) * 128],
                        qtc[c][:, k, sh * 256 : (sh + 1) * 256],
                        start=(k == 0),
                        stop=(k == NKT - 1),
                        skip_group_check=True,
                    )
                return f

            def epi():
                nc.vector.tensor_scalar_add(
                    dst[:, s0 : s0 + 256], st.pop("t"), b_t[:, pair : pair + 1]
                )

            return [mm(k) for k in range(NKT)] + [epi]

        def emit_pv_half(c, i):
            """v for s-tile 4c+i: 8 matmuls + the v_ext copy epilogue."""
            st = {}

            def mm(k):
                def f():
                    if k == 0:
                        st["t"] = pslot()
                    nc.tensor.matmul(
                        st["t"],
                        qtc[c][:, k, i * 128 : (i + 1) * 128],
                        wv_t[:, k, :],
                        start=(k == 0),
                        stop=(k == NKT - 1),
                        skip_group_check=True,
                    )
                return f

            def epi():
                nc.vector.tensor_copy(
                    v_ext[c * 4 + i][:, :, 0:64],
                    st.pop("t").rearrange("p (h d) -> p h d", h=4),
                )

            return [mm(k) for k in range(NKT)] + [epi]

        # ---------- prologue: minimum prefix (kT[0]/qT[0] chunk 0) ----------
        # four half-chains; q rides the (still idle) ctx arena so k/q can
        # interleave PER UNIT and fill the PE's waits between qtc0 k-tile
        # DMA arrivals. Units sharing a PSUM bank must NOT interleave their
        # matmuls -- two open accumulation groups in one bank corrupt it --
        # so the interleave is k-unit/q-unit (different banks), never ka/kb.
        _ka = emit_qk_half(0, 0, "k", 0)
        _kb = emit_qk_half(0, 0, "k", 1)
        _qa = emit_qk_half(0, 0, "q", 0, slot_ap=ctx_arena[:, 0:256])
        _qb = emit_qk_half(0, 0, "q", 1, slot_ap=ctx_arena[:, 256:512])
        for k in range(NKT):
            _ka[k]()
            _qa[k]()
        for k in range(NKT):
            _kb[k]()
            _qb[k]()
        for th in (_ka[8], _kb[8], _qa[8], _qb[8]):
            th()

        # ---------- filler: remaining projections, deadline-paced ----------
        chains = [
            (emit_pv_half(0, 0), 0),
            (emit_pv_half(0, 1), 0.5),
            (emit_pv_half(0, 2), 1),
            (emit_pv_half(0, 3), 1.5),
            (emit_qk_half(0, 1, "k", 0), 2.5),
            (emit_qk_half(0, 1, "k", 1), 3),
            (emit_pv_half(1, 0), 4),
            (emit_pv_half(1, 1), 4.5),
            (emit_pv_half(1, 2), 5),
            (emit_pv_half(1, 3), 5.5),
            (emit_qk_half(0, 2, "k", 0), 6.5),
            (emit_qk_half(0, 2, "k", 1), 7),
            (emit_pv_half(2, 0), 8),
            (emit_pv_half(2, 1), 8.5),
            (emit_pv_half(2, 2), 9),
            (emit_pv_half(2, 3), 9.5),
            (emit_qk_half(0, 3, "k", 0), 10.5),
            (emit_qk_half(0, 3, "k", 1), 11),
            (emit_pv_half(3, 0), 12),
            (emit_pv_half(3, 1), 12.5),
            (emit_pv_half(3, 2), 13),
            (emit_pv_half(3, 3), 13.5),
            (emit_qk_half(0, 1, "q", 0), 14.5),
            (emit_qk_half(0, 1, "q", 1), 15),
            (emit_qk_half(0, 2, "q", 0), 28),
            (emit_qk_half(0, 2, "q", 1), 30),
            (emit_qk_half(0, 3, "q", 0), 42),
            (emit_qk_half(0, 3, "q", 1), 44),
            (emit_qk_half(1, 0, "k", 0), 54),
            (emit_qk_half(1, 0, "k", 1), 56),
            (emit_qk_half(1, 0, "q", 0), 58),
            (emit_qk_half(1, 0, "q", 1), 60),
            (emit_qk_half(1, 1, "k", 0), 62),
            (emit_qk_half(1, 1, "k", 1), 64),
            (emit_qk_half(1, 2, "k", 0), 68),
            (emit_qk_half(1, 2, "k", 1), 70),
            (emit_qk_half(1, 3, "k", 0), 72),
            (emit_qk_half(1, 3, "k", 1), 73),
            (emit_qk_half(1, 1, "q", 0), 75),
            (emit_qk_half(1, 1, "q", 1), 76),
            (emit_qk_half(1, 2, "q", 0), 88),
            (emit_qk_half(1, 2, "q", 1), 90),
            (emit_qk_half(1, 3, "q", 0), 104),
            (emit_qk_half(1, 3, "q", 1), 106),
        ]
        # units pop ATOMICALLY (all 9 thunks at once): interleaving the
        # matmuls of two units that share the proj PSUM bank would leave two
        # accumulation groups open in one bank at once, which corrupts it
        work = deque()
        for thunks, dl in chains:
            work.append((dl, thunks))

        def fill_slot(i):
            while work and work[0][0] <= i:
                for th in work.popleft()[1]:
                    th()

        # ---------- attention: one continuous 128-step pipeline ----------
        blocks = [(p, sc) for p in range(2) for sc in range(NSC)]
        NB = len(blocks)

        # Chains come in (head0, head1) pairs for the same s-subtile; the pair
        # lands in adjacent ctx_arena slots so ONE strided reciprocal covers
        # both denominators, then two scalar_tensor_tensor epilogues and the
        # out-tile half DMAs.
        pair_rot = [0]
        pts = {}

        def issue_chain(b, j):
            """AV chain j (= 2*si + head) of block b: 16 matmuls; epilogue for
            both heads of the s-subtile after the odd (second) chain."""
            pair, sc = blocks[b]
            si, hl = divmod(j, 2)
            h_abs = 2 * pair + hl
            blk = sc * 4 + si
            sl = 2 * (pair_rot[0] % 3) + hl
            if hl == 1:
                pair_rot[0] += 1
            slot = ctx_arena[:, 65 * sl : 65 * sl + 65]
            for t in range(NTT):
                nc.tensor.matmul(
                    slot,
                    pts[16 * b + t][:, hl * 512 + si * 128 : hl * 512 + (si + 1) * 128],
                    v_ext[t][:, h_abs, :],
                    start=(t == 0),
                    stop=(t == NTT - 1),
                    skip_group_check=True,
                )
            if hl == 0:
                return
            # epilogue for both chains of the pair (slots sl-1, sl)
            a0 = ctx_arena[:, 65 * (sl - 1) + 64 : 65 * (sl - 1) + 65]
            den2 = bass.AP(tensor=a0.tensor, offset=a0.offset, ap=[a0.ap[0], [65, 2]])
            r2 = rp.tile([128, 2], F32, name="r", tag="r")
            nc.vector.reciprocal(r2[:], den2)
            for hh in range(2):
                ha = 2 * pair + hh
                nc.vector.scalar_tensor_tensor(
                    out=out_tiles[blk][:, ha * 64 : (ha + 1) * 64],
                    in0=ctx_arena[:, 65 * (sl - 1 + hh) : 65 * (sl - 1 + hh) + 64],
                    scalar=r2[:, hh : hh + 1],
                    in1=bv_t[:, ha * 64 : (ha + 1) * 64],
                    op0=mybir.AluOpType.mult,
                    op1=mybir.AluOpType.add,
                )
            jsl = slice(pair * 128, pair * 128 + 128)
            nc.sync.dma_start(
                out[blk * 128 : (blk + 1) * 128, jsl], out_tiles[blk][:, jsl]
            )

        for i in range(NB * NTT):
            bb, tt = divmod(i, NTT)
            pair, sc = blocks[bb]
            s0 = sc * 512
            qTt, kTt = qT[pair], kT[pair]
            tsl = slice(tt * 128, (tt + 1) * 128)
            # both heads' scoresT share one 2-bank tile; the exp is split into
            # two per-head [128, 512] halves spread across ACT and DVE (~19:13)
            # so neither engine paces the PE -- a whole-tile exp on one engine
            # (1.1us) is slower than a PE step and was the v3 bottleneck
            g = aps.tile([128, 1024], F32, name="g", tag="grp", bufs=3)
            nc.tensor.matmul(
                g[:, 0:512],
                kTt[0:64, tsl],
                qTt[0:64, s0 : s0 + 512],
                start=True,
                stop=True,
                tile_position=(0, 0),
            )
            nc.tensor.matmul(
                g[:, 512:1024],
                kTt[64:128, tsl],
                qTt[64:128, s0 : s0 + 512],
                start=True,
                stop=True,
                tile_position=(64, 0),
            )
            pT_ = pTp.tile([128, 1024], BF16, name="pT_", tag="pT")
            if tt % 8 in (1, 4, 6):
                nc.vector.tensor_scalar(
                    out=pT_[:].bitcast(U16),
                    in0=g[:],
                    scalar1=SCH_A,
                    scalar2=SCH_B,
                    op0=mybir.AluOpType.mult,
                    op1=mybir.AluOpType.add,
                )
            else:
                nc.scalar.activation(
                    pT_[:], g[:],
                    mybir.ActivationFunctionType.Exp, scale=0.125,
                )
            pts[i] = pT_
            # filler between the exp and this step's AV chain: the PE chews
            # projections instead of stalling
            fill_slot(i)
            if bb >= 1 and tt % 2 == 1:
                issue_chain(bb - 1, tt // 2)
                if tt == NTT - 1:
                    for t in range(NTT):
                        del pts[16 * (bb - 1) + t]

        # drain: leftover filler, then the last block's AV chains
        while work:
            work.popleft()[1]()
        for j in range(8):
            issue_chain(NB - 1, j)

    nc.compile()
    return nc


def kernel(Q, Wq, bq, Wk, bk, Wv, bv):
    global _cached_nc, last_result
    Q = np.asarray(Q, dtype=np.float32)
    Wq, Wk, Wv = (np.asarray(w, dtype=np.float32) for w in (Wq, Wk, Wv))
    bq, bk, bv = (np.asarray(b, dtype=np.float32) for b in (bq, bk, bv))
    B = Q.shape[0]
    assert Q.shape == (B, S, D) and B * 4 == N_CORES

    if _cached_nc is None:
        _cached_nc = _build()
    nc = _cached_nc

    # host-side shard prep (bf16 inputs: full PE rate, half the DMA traffic)
    bf16 = ml_dtypes.bfloat16
    qts = [np.ascontiguousarray(Q[b].T).astype(bf16) for b in range(B)]
    wqs = [np.ascontiguousarray(Wq[g * J : (g + 1) * J, :].T).astype(bf16) for g in range(4)]
    wks = [np.ascontiguousarray(Wk[g * J : (g + 1) * J, :].T).astype(bf16) for g in range(4)]
    wvs = [np.ascontiguousarray(Wv[g * J : (g + 1) * J, :].T).astype(bf16) for g in range(4)]

    in_maps = []
    for c in range(N_CORES):
        b, g = c // 4, c % 4
        jsl = slice(g * J, (g + 1) * J)
        in_maps.append(
            {
                "qt": qts[b],
                "wq": wqs[g],
                "wk": wks[g],
                "wv": wvs[g],
                "bq": np.ascontiguousarray(bq[jsl]),
                "bk": np.ascontiguousarray(bk[jsl]),
                "bv": np.ascontiguousarray(bv[jsl]),
            }
        )

    last_result = run_bass_kernel_spmd(nc, in_maps, list(range(N_CORES)))

    full = np.empty((B, S, D), dtype=np.float32)
    for c in range(N_CORES):
        b, g = c // 4, c % 4
        full[b, :, g * J : (g + 1) * J] = last_result.results[c]["out"]
    return full
